# revision 1
# baseline (speedup 1.0000x reference)
"""Trainium2 Bass kernel for nn_DigitConvolutionalModel.

Model: x[B,784] -> conv3x3(valid, 28x28->26x26) -> flatten -> Linear(676,256)
       -> relu -> Linear(256,10).

The conv is linear, so it is folded into the first Linear on the host:
  h_pre = x @ W1eff + b1,  W1eff[784,256] = C @ W1.T  (C = conv as matrix)
leaving a plain 2-layer MLP for the device:
  out = relu(x @ W1eff + b1) @ W2.T + b2

Sharding: pure data parallelism over the batch dim across 8 NeuronCores
(8192 samples/core); weights replicated.

Numerics: x is transported in fp8 E3M4 (4 mantissa bits; the PE streams
fp8 moving data at the same 1 col/cycle as bf16, so this halves HBM
traffic at zero PE cost); weights stay bf16, accumulation fp32 in PSUM.
Measured rel err 0.0132 vs the 0.02 gate.

Schedule: the 784-dim contraction is 6 full 128-row chunks plus a
16-row tail applied via 4 row-tiled matmuls packed into distinct 32-row
PE groups (they execute concurrently, ~4ns apart). Batch groups run in
blocks of 2 (4 layer-1 PSUM banks per block from a 6-bank ps1 pool).
Per block: both k-loops, then the tail pack (so relus start early),
then the previous block's deferred layer-2 flushes (their h inputs are
long ready by then). Engine roles are strict to avoid cross-block FIFO
dependency cycles: scalar = relus only, vector = bias adds only,
sync ring = all x/xt loads (need-order, group 0 in contiguous column
quarters), scalar ring = weight slices in consumption order, gpsimd
SWDGE = interior output stores, final stores on the idle sync ring.
Warmup/filler matmuls keep the PE busy through the startup DMA fill so
the HAM clock gate un-throttles early, and block 1's PSUM banks are
zero-primed to bridge the block-0 -> block-1 handoff.
"""

import sys

if "/opt/trn_rl_repo" not in sys.path:
    sys.path.insert(0, "/opt/trn_rl_repo")

import ml_dtypes
import numpy as np

f8 = ml_dtypes.float8_e3m4

B = 65536
NCORES = 8
BC = B // NCORES  # 8192 samples per core
P = 128
KM = 6            # full 128-row contraction chunks (rows 0..767)
KT = 16           # tail contraction rows (768..783)
NF1 = 256         # layer-1 output features (2 halves of 128)
NO = 10           # logits
NB = 512          # batch columns per matmul group (one PSUM bank, fp32)
NGRP = BC // NB   # 16 groups per core
NBLK = NGRP // 2  # 8 blocks of 2 groups
WCOLS = KM * NF1 + P + 2 * NO  # packed weight columns: w1 | w1tail | w2

_PROG = None


def _build_program():
    import concourse.tile as tile
    from concourse import bacc, mybir

    bf16 = mybir.dt.bfloat16
    f32 = mybir.dt.float32
    Relu = mybir.ActivationFunctionType.Relu
    add = mybir.AluOpType.add
    amax = mybir.AluOpType.max

    nc = bacc.Bacc("TRN2", target_bir_lowering=False, debug=False,
                   num_devices=NCORES)
    f8 = mybir.dt.float8e3
    xq0 = nc.dram_tensor("xq0", [P, 4, KM * 128], f8,
                         kind="ExternalInput").ap()
    xb = nc.dram_tensor("xb", [P, NGRP, KM, NB], f8,
                        kind="ExternalInput").ap()
    xtl = nc.dram_tensor("xtl", [P, NBLK, NB], f8,
                         kind="ExternalInput").ap()
    wp = nc.dram_tensor("wp", [P, WCOLS], bf16, kind="ExternalInput").ap()
    b1 = nc.dram_tensor("b1", [P, 2], f32, kind="ExternalInput").ap()
    b2 = nc.dram_tensor("b2", [NO, 1], f32, kind="ExternalInput").ap()
    out = nc.dram_tensor("out", [NO, BC], f32, kind="ExternalOutput").ap()

    with tile.TileContext(nc) as tc:
        with (
            tc.tile_pool(name="singles", bufs=1) as singles,
            tc.tile_pool(name="xp", bufs=7) as xp,
            tc.tile_pool(name="xtp", bufs=8) as xtp,
            tc.tile_pool(name="hp", bufs=8) as hp,
            tc.tile_pool(name="op", bufs=5) as op,
            tc.tile_pool(name="ps1", bufs=6, space="PSUM") as ps1p,
            tc.tile_pool(name="ps2", bufs=2, space="PSUM") as ps2p,
        ):
            # warmup tile + matmuls: keep the PE busy from the first cycle
            wsb = singles.tile([P, P], bf16)
            nc.vector.memset(wsb, 0.0)
            wmp = ps1p.tile([32, P], f32, tag="ps1", name="warm")

            def filler(n):
                for i in range(n):
                    nc.tensor.matmul(wmp, wsb[:, :32], wsb,
                                     start=(i == 0), stop=(i == n - 1))

            filler(24)

            # ---- weights on the scalar HWDGE ring, sliced in the exact
            # order the opening matmuls consume them; the second eighth of
            # group 0 also rides this ring so the two rings fill the first
            # PSUM banks in parallel.
            wpsb = singles.tile([P, WCOLS], bf16)
            W1T0 = KM * NF1
            W20 = W1T0 + P

            def w1w(k, m):
                c = k * NF1 + m * P
                return wpsb[:, c:c + P]

            def w2w(m):
                c = W20 + m * NO
                return wpsb[:, c:c + NO]

            nc.scalar.dma_start(out=wpsb[:, 0:512], in_=wp[:, 0:512])
            nc.scalar.dma_start(out=wpsb[:, 512:1024], in_=wp[:, 512:1024])
            nc.scalar.dma_start(out=wpsb[:, 1024:WCOLS],
                                in_=wp[:, 1024:WCOLS])
            b1sb = singles.tile([P, 2], f32)
            nc.scalar.dma_start(out=b1sb, in_=b1)
            b2sb = singles.tile([NO, 1], f32)
            nc.scalar.dma_start(out=b2sb, in_=b2)

            # ---- x on the sync ring: group 0 in contiguous column
            # quarters, then whole groups
            xg0sb = singles.tile([P, 4, KM, 128], f8)
            for q in range(4):
                nc.sync.dma_start(out=xg0sb[:, q], in_=xq0[:, q])
            xtiles = [None] * NGRP
            xttiles = [None] * NBLK

            def load_group(g):
                # all x on the sync ring: fp8 transport halved the bytes,
                # so one ring has ample slack, and the scalar queue stays a
                # pure compute pipeline (late dispatches there delay relus)
                xg = xp.tile([P, KM, NB], f8, tag="x", name=f"x_{g}")
                nc.sync.dma_start(out=xg, in_=xb[:, g])
                xtiles[g] = xg

            def load_tail(b):
                xt = xtp.tile([P, NB], f8, tag="xt", name=f"xt_{b}")
                nc.sync.dma_start(out=xt, in_=xtl[:, b])
                xttiles[b] = xt

            load_group(1)
            load_tail(0)
            load_tail(1)
            load_group(2)
            load_group(3)

            osbs = [None] * NBLK

            def layer2(hs, g):
                blk, j = g // 2, g % 2
                if osbs[blk] is None:
                    osbs[blk] = op.tile([NO, 2, NB], f32, tag="o",
                                        name=f"o_{blk}")
                osb = osbs[blk]
                ps2 = ps2p.tile([NO, NB], f32, tag="ps2", name=f"ps2_{g}")
                for m in range(2):
                    nc.tensor.matmul(ps2, w2w(m), hs[m],
                                     start=(m == 0), stop=(m == 1))
                # bias on vector, relus on scalar: an engine that runs both
                # forms a cross-block dependency cycle (bias waits on L2
                # matmuls which wait on relus queued behind the bias)
                nc.vector.tensor_scalar_add(osb[:, j], ps2, b2sb)
                if blk == NBLK - 1:
                    gs = slice(g * NB, (g + 1) * NB)
                    nc.sync.dma_start(out=out[:, gs], in_=osb[:, j])
                elif j == 1:
                    # interior stores ride the software DGE on the idle
                    # gpsimd queue (one 40KB store per ~5us keeps up)
                    gs = slice(blk * 2 * NB, (blk + 1) * 2 * NB)
                    nc.gpsimd.dma_start(out=out[:, gs], in_=osb)

            def relu_pair(g, pss):
                # both relus on the scalar engine (its only job)
                hs = []
                for m in range(2):
                    h = hp.tile([P, NB], bf16, tag="h", name=f"h_{g}_{m}")
                    nc.scalar.activation(h, pss[(g, m)], Relu,
                                         bias=b1sb[:, m:m + 1])
                    hs.append(h)
                return hs

            def pack_pair(g, pss):
                # 16-row contraction tail for one group: 2 row-tiled
                # matmuls in distinct 32-row PE groups (concurrent)
                xt = xttiles[g // 2]
                base = 0 if g % 2 == 0 else 64
                for m in range(2):
                    rs = slice(base + 32 * m, base + 32 * m + KT)
                    nc.tensor.matmul(pss[(g, m)], wpsb[rs, W1T0:W1T0 + P],
                                     xt[rs], start=False, stop=True,
                                     tile_position=(base + 32 * m, 0))

            def pack_quad(blk, pss):
                g0, g1 = 2 * blk, 2 * blk + 1
                xt = xttiles[blk]
                for r, (g, m) in enumerate(
                        [(g0, 0), (g0, 1), (g1, 0), (g1, 1)]):
                    rs = slice(32 * r, 32 * r + KT)
                    nc.tensor.matmul(pss[(g, m)], wpsb[rs, W1T0:W1T0 + P],
                                     xt[rs], start=False, stop=True,
                                     tile_position=(32 * r, 0))

            pend = []
            # ================= block 0 (groups 0, 1) =================
            pss = {(g, m): ps1p.tile([P, NB], f32, tag="ps1",
                                     name=f"ps1_{g}_{m}")
                   for g in (0, 1) for m in range(2)}
            started = set()

            def mm0(q, m, ks):
                for k in ks:
                    cs = slice(q * 128, (q + 1) * 128)
                    st = (0, m) not in started
                    started.add((0, m))
                    nc.tensor.matmul(pss[(0, m)][:, cs], w1w(k, m),
                                     xg0sb[:, q, k], start=st, stop=False)

            # phases ordered by DMA arrival (weights wA=k0k1/wB=k2k3/wC;
            # group-0 quarters q0-q3); fillers bridge the known arrival
            # bubbles so the HAM clock gate never sees a PE-idle window
            for m in range(2):
                mm0(0, m, [0, 1])         # wA + q0
            filler(6)
            for m in range(2):
                mm0(1, m, [0, 1])         # q1
            for q in (0, 1):
                for m in range(2):
                    mm0(q, m, [2, 3])     # wB
            for m in range(2):
                mm0(2, m, range(4))       # q2
            for q in (0, 1):
                for m in range(2):
                    mm0(q, m, [4, 5])     # wC
            for m in range(2):
                mm0(3, m, range(4))       # q3
            for q in (2, 3):
                for m in range(2):
                    mm0(q, m, [4, 5])
            filler(4)
            # group 1 whole (N=512)
            for m in range(2):
                for k in range(KM):
                    nc.tensor.matmul(pss[(1, m)], w1w(k, m),
                                     xtiles[1][:, k],
                                     start=(k == 0), stop=False)
            load_group(4)
            load_group(5)
            load_tail(2)
            pack_quad(0, pss)
            pend.append((relu_pair(0, pss), 0))
            pend.append((relu_pair(1, pss), 1))

            # ================= blocks 1..7 =================
            for blk in range(1, NBLK):
                g0, g1 = 2 * blk, 2 * blk + 1
                if 2 * blk + 4 < NGRP:
                    load_group(2 * blk + 4)
                    load_group(2 * blk + 5)
                    if blk + 2 < NBLK:
                        load_tail(blk + 2)

                pss = {(g, m): ps1p.tile([P, NB], f32, tag="ps1",
                                         name=f"ps1_{g}_{m}")
                       for g in (g0, g1) for m in range(2)}
                primed = set()
                if blk == 1:
                    # zero-prime all 4 banks: 0-weight matmuls keep the PE
                    # busy across the block-0 -> block-1 DMA handoff and
                    # leave the banks cleared for accumulation
                    for g in (g0, g1):
                        for m in range(2):
                            nc.tensor.matmul(pss[(g, m)], wsb,
                                             xtiles[1][:, 0],
                                             start=True, stop=False)
                            primed.add((g, m))

                def kloop(g):
                    for m in range(2):
                        for k in range(KM):
                            nc.tensor.matmul(
                                pss[(g, m)], w1w(k, m), xtiles[g][:, k],
                                start=(k == 0 and (g, m) not in primed),
                                stop=False)

                if blk < NBLK - 1:
                    # pack right after the k-loops so the relus start as
                    # early as possible; the deferred layer-2 flushes run
                    # after (their h inputs are then long ready)
                    kloop(g0)
                    kloop(g1)
                    pack_quad(blk, pss)
                    while pend:
                        layer2(*pend.pop(0))
                    pend.append((relu_pair(g0, pss), g0))
                    pend.append((relu_pair(g1, pss), g1))
                else:
                    # last block: per-group tails so the final store chain
                    # is short; g14's relu runs during g15's k-loop
                    kloop(g0)
                    pack_pair(g0, pss)
                    layer2(*pend.pop(0))
                    hs14 = relu_pair(g0, pss)
                    kloop(g1)
                    pack_pair(g1, pss)
                    # final group's relus split across scalar/vector and
                    # emitted BEFORE the layer-2 flushes: they run in
                    # parallel with the other groups' layer-2 matmuls, so
                    # the closing chain is one activation latency
                    h0 = hp.tile([P, NB], bf16, tag="h", name=f"h_{g1}_0")
                    nc.scalar.activation(h0, pss[(g1, 0)], Relu,
                                         bias=b1sb[:, 0:1])
                    h1 = hp.tile([P, NB], bf16, tag="h", name=f"h_{g1}_1")
                    nc.vector.tensor_scalar(h1, pss[(g1, 1)], b1sb[:, 1:2],
                                            0.0, add, amax)
                    layer2(*pend.pop(0))
                    layer2(hs14, g0)
                    layer2([h0, h1], g1)

    nc.compile()
    return nc


def _fold_weights(conv_w, W1):
    """W1eff[784,256] such that x @ W1eff == flatten(conv(x)) @ W1.T."""
    cw = conv_w.astype(np.float64)
    W1r = W1.astype(np.float64).reshape(NF1, 26, 26).transpose(1, 2, 0)
    W1eff = np.zeros((28, 28, NF1), np.float64)
    for dr in range(3):
        for dc in range(3):
            W1eff[dr:dr + 26, dc:dc + 26, :] += cw[dr, dc] * W1r
    return W1eff.reshape(784, NF1)


def _prep_inputs(x, conv_w, W1, b1, W2, b2):
    bf16 = ml_dtypes.bfloat16
    W1eff = _fold_weights(conv_w, W1)
    wpack = np.zeros((P, WCOLS), np.float64)
    wpack[:, :KM * NF1] = W1eff[:768].reshape(KM, P, NF1).transpose(
        1, 0, 2).reshape(P, KM * NF1)
    W1T0 = KM * NF1
    wpack[0:KT, W1T0:W1T0 + P] = W1eff[768:784, 0:128]
    wpack[32:32 + KT, W1T0:W1T0 + P] = W1eff[768:784, 128:256]
    wpack[64:64 + KT, W1T0:W1T0 + P] = W1eff[768:784, 0:128]
    wpack[96:96 + KT, W1T0:W1T0 + P] = W1eff[768:784, 128:256]
    W20 = W1T0 + P
    wpack[:, W20:W20 + 2 * NO] = W2.T.astype(np.float64).reshape(
        2, P, NO).transpose(1, 0, 2).reshape(P, 2 * NO)
    wpack = wpack.astype(bf16)
    b1p = np.ascontiguousarray(b1.astype(np.float32).reshape(2, P).T)
    b2p = b2.astype(np.float32).reshape(NO, 1)

    in_maps = []
    for c in range(NCORES):
        xcT = np.ascontiguousarray(
            x[c * BC:(c + 1) * BC].T).astype(f8)  # [784, BC]
        xmain = np.ascontiguousarray(
            xcT[:768].reshape(KM, P, NGRP, NB).transpose(1, 2, 0, 3))
        xq0p = np.ascontiguousarray(
            xcT[:768, 0:512].reshape(KM, P, 4, 128).transpose(1, 2, 0, 3)
        ).reshape(P, 4, KM * 128)
        xtail = np.zeros((P, NBLK, NB), f8)
        tl = xcT[768:784].reshape(KT, NBLK, 2, NB)
        xtail[0:KT] = tl[:, :, 0]
        xtail[32:32 + KT] = tl[:, :, 0]
        xtail[64:64 + KT] = tl[:, :, 1]
        xtail[96:96 + KT] = tl[:, :, 1]
        in_maps.append({
            "xq0": np.ascontiguousarray(xq0p),
            "xb": xmain, "xtl": xtail,
            "wp": wpack, "b1": b1p, "b2": b2p,
        })
    return in_maps


def kernel(x, conv_w, W1, b1, W2, b2, _trace=False, _trace_kwargs=None):
    global _PROG
    from concourse import bass_utils

    x = np.asarray(x, dtype=np.float32)
    conv_w = np.asarray(conv_w, dtype=np.float32)
    W1 = np.asarray(W1, dtype=np.float32)
    b1 = np.asarray(b1, dtype=np.float32)
    W2 = np.asarray(W2, dtype=np.float32)
    b2 = np.asarray(b2, dtype=np.float32)
    assert x.shape == (B, 784), x.shape

    if _PROG is None:
        _PROG = _build_program()

    in_maps = _prep_inputs(x, conv_w, W1, b1, W2, b2)
    kwargs = dict(_trace_kwargs or {})
    res = bass_utils.run_bass_kernel_spmd(
        _PROG, in_maps, core_ids=list(range(NCORES)), trace=_trace, **kwargs)

    out = np.empty((B, NO), np.float32)
    for c in range(NCORES):
        out[c * BC:(c + 1) * BC] = res.results[c]["out"].T
    if _trace:
        return out, res
    return out



# revision 5
# speedup vs baseline: 1.1072x; 1.1072x over previous
"""Trainium2 Bass kernel for nn_DigitConvolutionalModel.

Model: x[B,784] -> conv3x3(valid, 28x28->26x26) -> flatten -> Linear(676,256)
       -> relu -> Linear(256,10).

The conv is linear, so it is folded into the first Linear on the host:
  h_pre = x @ W1eff + b1,  W1eff[784,256] = C @ W1.T  (C = conv as matrix)
leaving a plain 2-layer MLP for the device:
  out = relu(x @ W1eff + b1) @ W2.T + b2

Sharding: pure data parallelism over the batch dim across 8 NeuronCores
(8192 samples/core); weights replicated.

Numerics: x is transported in fp8 E3M4 (4 mantissa bits; the PE streams
fp8 moving data at the same 1 col/cycle as bf16, so this halves HBM
traffic at zero PE cost); weights stay bf16, accumulation fp32 in PSUM.
Measured rel err 0.0132 vs the 0.02 gate.

Schedule: the 784-dim contraction is 6 full 128-row chunks plus a
16-row tail applied via 4 row-tiled matmuls packed into distinct 32-row
PE groups (they execute concurrently, ~4ns apart). Batch groups run in
blocks of 2 (4 layer-1 PSUM banks per block from a 6-bank ps1 pool).
Per block: both k-loops, then the tail pack (so relus start early),
then the previous block's deferred layer-2 flushes (their h inputs are
long ready by then). Engine roles are strict to avoid cross-block FIFO
dependency cycles: scalar = relus only, vector = bias adds only,
sync ring = all x/xt loads (need-order, group 0 in contiguous column
quarters), scalar ring = weight slices in consumption order, gpsimd
SWDGE = interior output stores, final stores on the idle sync ring.
Warmup/filler matmuls keep the PE busy through the startup DMA fill so
the HAM clock gate un-throttles early, and block 1's PSUM banks are
zero-primed to bridge the block-0 -> block-1 handoff.
"""

import sys

if "/opt/trn_rl_repo" not in sys.path:
    sys.path.insert(0, "/opt/trn_rl_repo")

import ml_dtypes
import numpy as np

f8 = ml_dtypes.float8_e3m4

B = 65536
NCORES = 8
BC = B // NCORES  # 8192 samples per core
P = 128
KM = 6            # full 128-row contraction chunks (rows 0..767)
KT = 16           # tail contraction rows (768..783)
NF1 = 256         # layer-1 output features (2 halves of 128)
NO = 10           # logits
NB = 512          # batch columns per matmul group (one PSUM bank, fp32)
NGRP = BC // NB   # 16 groups per core
NBLK = NGRP // 2  # 8 blocks of 2 groups
WCOLS = KM * NF1 + P + 2 * NO  # packed weight columns: w1 | w1tail | w2

_PROG = None


def _build_program():
    import concourse.tile as tile
    from concourse import bacc, mybir

    bf16 = mybir.dt.bfloat16
    f32 = mybir.dt.float32
    Relu = mybir.ActivationFunctionType.Relu
    add = mybir.AluOpType.add
    amax = mybir.AluOpType.max

    nc = bacc.Bacc("TRN2", target_bir_lowering=False, debug=False,
                   num_devices=NCORES)
    f8 = mybir.dt.float8e3
    # group-contiguous layouts: each slice below is one fully sequential
    # HBM read (partition-major inside), which keeps the SDMA engines on
    # contiguous streams
    xq0 = nc.dram_tensor("xq0", [4, P, KM * 128], f8,
                         kind="ExternalInput").ap()
    xb = nc.dram_tensor("xb", [NGRP, P, KM * NB], f8,
                        kind="ExternalInput").ap()
    xtl = nc.dram_tensor("xtl", [NBLK, P, NB], f8,
                         kind="ExternalInput").ap()
    wp = nc.dram_tensor("wp", [P, WCOLS], bf16, kind="ExternalInput").ap()
    b1 = nc.dram_tensor("b1", [P, 2], f32, kind="ExternalInput").ap()
    b2 = nc.dram_tensor("b2", [NO, 1], f32, kind="ExternalInput").ap()
    out = nc.dram_tensor("out", [NO, BC], f32, kind="ExternalOutput").ap()

    with tile.TileContext(nc) as tc:
        with (
            tc.tile_pool(name="singles", bufs=1) as singles,
            tc.tile_pool(name="xp", bufs=10) as xp,
            tc.tile_pool(name="xtp", bufs=8) as xtp,
            tc.tile_pool(name="hp", bufs=8) as hp,
            tc.tile_pool(name="op", bufs=5) as op,
            tc.tile_pool(name="ps1", bufs=6, space="PSUM") as ps1p,
            tc.tile_pool(name="ps2", bufs=2, space="PSUM") as ps2p,
        ):
            # warmup tile + matmuls: keep the PE busy from the first cycle
            wsb = singles.tile([P, P], bf16)
            nc.vector.memset(wsb, 0.0)
            wmp = ps1p.tile([32, P], f32, tag="ps1", name="warm")

            def filler(n):
                for i in range(n):
                    nc.tensor.matmul(wmp, wsb[:, :32], wsb,
                                     start=(i == 0), stop=(i == n - 1))

            filler(24)

            # ---- weights on the scalar HWDGE ring, sliced in the exact
            # order the opening matmuls consume them; the second eighth of
            # group 0 also rides this ring so the two rings fill the first
            # PSUM banks in parallel.
            wpsb = singles.tile([P, WCOLS], bf16)
            W1T0 = KM * NF1
            W20 = W1T0 + P

            def w1w(k, m):
                c = k * NF1 + m * P
                return wpsb[:, c:c + P]

            def w2w(m):
                c = W20 + m * NO
                return wpsb[:, c:c + NO]

            nc.scalar.dma_start(out=wpsb[:, 0:512], in_=wp[:, 0:512])
            nc.scalar.dma_start(out=wpsb[:, 512:1024], in_=wp[:, 512:1024])
            nc.scalar.dma_start(out=wpsb[:, 1024:WCOLS],
                                in_=wp[:, 1024:WCOLS])
            b1sb = singles.tile([P, 2], f32)
            nc.scalar.dma_start(out=b1sb, in_=b1)
            b2sb = singles.tile([NO, 1], f32)
            nc.scalar.dma_start(out=b2sb, in_=b2)

            # ---- x split across BOTH hardware DGE rings (sync + scalar):
            # one ring alone saturates at ~95-200 GB/s, below the ~266 GB/s
            # the PE needs at roofline. Group 0 quarters + g1 on sync (the
            # scalar ring is still busy with the weight prologue), then
            # even groups ride scalar, odd groups ride sync.
            xg0sb = singles.tile([P, 4, KM, 128], f8)
            for q in range(4):
                nc.sync.dma_start(out=xg0sb[:, q], in_=xq0[q])
            xtiles = [None] * NGRP
            xttiles = [None] * NBLK

            def load_group(g):
                xg = xp.tile([P, KM, NB], f8, tag="x", name=f"x_{g}")
                ring = nc.sync if (g == 1 or g % 2 == 1) else nc.scalar
                ring.dma_start(out=xg, in_=xb[g])
                xtiles[g] = xg

            def load_tail(b):
                # xtp has NBLK bufs (no recycling), so these dma_starts on
                # the scalar queue never block behind a pool wait
                xt = xtp.tile([P, NB], f8, tag="xt", name=f"xt_{b}")
                nc.scalar.dma_start(out=xt, in_=xtl[b])
                xttiles[b] = xt

            load_group(1)
            load_tail(0)
            load_tail(1)
            load_group(2)
            load_group(3)

            osbs = [None] * NBLK

            def layer2(hs, g):
                blk, j = g // 2, g % 2
                if osbs[blk] is None:
                    osbs[blk] = op.tile([NO, 2, NB], f32, tag="o",
                                        name=f"o_{blk}")
                osb = osbs[blk]
                ps2 = ps2p.tile([NO, NB], f32, tag="ps2", name=f"ps2_{g}")
                for m in range(2):
                    nc.tensor.matmul(ps2, w2w(m), hs[m],
                                     start=(m == 0), stop=(m == 1))
                # bias on vector, relus on scalar: an engine that runs both
                # forms a cross-block dependency cycle (bias waits on L2
                # matmuls which wait on relus queued behind the bias)
                nc.vector.tensor_scalar_add(osb[:, j], ps2, b2sb)
                if blk == NBLK - 1:
                    gs = slice(g * NB, (g + 1) * NB)
                    nc.sync.dma_start(out=out[:, gs], in_=osb[:, j])
                elif j == 1:
                    # interior stores ride the software DGE on the idle
                    # gpsimd queue (one 40KB store per ~5us keeps up)
                    gs = slice(blk * 2 * NB, (blk + 1) * 2 * NB)
                    nc.gpsimd.dma_start(out=out[:, gs], in_=osb)

            def relu_pair(g, pss):
                # both relus on the scalar engine (its only job)
                hs = []
                for m in range(2):
                    h = hp.tile([P, NB], bf16, tag="h", name=f"h_{g}_{m}")
                    nc.scalar.activation(h, pss[(g, m)], Relu,
                                         bias=b1sb[:, m:m + 1])
                    hs.append(h)
                return hs

            def pack_pair(g, pss):
                # 16-row contraction tail for one group: 2 row-tiled
                # matmuls in distinct 32-row PE groups (concurrent)
                xt = xttiles[g // 2]
                base = 0 if g % 2 == 0 else 64
                for m in range(2):
                    rs = slice(base + 32 * m, base + 32 * m + KT)
                    nc.tensor.matmul(pss[(g, m)], wpsb[rs, W1T0:W1T0 + P],
                                     xt[rs], start=False, stop=True,
                                     tile_position=(base + 32 * m, 0))

            def pack_quad(blk, pss):
                g0, g1 = 2 * blk, 2 * blk + 1
                xt = xttiles[blk]
                for r, (g, m) in enumerate(
                        [(g0, 0), (g0, 1), (g1, 0), (g1, 1)]):
                    rs = slice(32 * r, 32 * r + KT)
                    nc.tensor.matmul(pss[(g, m)], wpsb[rs, W1T0:W1T0 + P],
                                     xt[rs], start=False, stop=True,
                                     tile_position=(32 * r, 0))

            pend = []
            # ================= block 0 (groups 0, 1) =================
            pss = {(g, m): ps1p.tile([P, NB], f32, tag="ps1",
                                     name=f"ps1_{g}_{m}")
                   for g in (0, 1) for m in range(2)}
            started = set()

            def mm0(q, m, ks):
                for k in ks:
                    cs = slice(q * 128, (q + 1) * 128)
                    st = (0, m) not in started
                    started.add((0, m))
                    nc.tensor.matmul(pss[(0, m)][:, cs], w1w(k, m),
                                     xg0sb[:, q, k], start=st, stop=False)

            # phases ordered by DMA arrival (weights wA=k0k1/wB=k2k3/wC;
            # group-0 quarters q0-q3); fillers bridge the known arrival
            # bubbles so the HAM clock gate never sees a PE-idle window
            for m in range(2):
                mm0(0, m, [0, 1])         # wA + q0
            filler(6)
            for m in range(2):
                mm0(1, m, [0, 1])         # q1
            for q in (0, 1):
                for m in range(2):
                    mm0(q, m, [2, 3])     # wB
            for m in range(2):
                mm0(2, m, range(4))       # q2
            for q in (0, 1):
                for m in range(2):
                    mm0(q, m, [4, 5])     # wC
            for m in range(2):
                mm0(3, m, range(4))       # q3
            for q in (2, 3):
                for m in range(2):
                    mm0(q, m, [4, 5])
            filler(4)
            # group 1 whole (N=512)
            for m in range(2):
                for k in range(KM):
                    nc.tensor.matmul(pss[(1, m)], w1w(k, m),
                                     xtiles[1][:, k],
                                     start=(k == 0), stop=False)
            load_group(4)
            load_group(5)
            load_tail(2)
            pack_quad(0, pss)
            pend.append((relu_pair(0, pss), 0))
            pend.append((relu_pair(1, pss), 1))

            # ================= blocks 1..7 =================
            for blk in range(1, NBLK):
                g0, g1 = 2 * blk, 2 * blk + 1
                if 2 * blk + 4 < NGRP:
                    load_group(2 * blk + 4)
                    load_group(2 * blk + 5)
                    if blk + 2 < NBLK:
                        load_tail(blk + 2)

                pss = {(g, m): ps1p.tile([P, NB], f32, tag="ps1",
                                         name=f"ps1_{g}_{m}")
                       for g in (g0, g1) for m in range(2)}
                primed = set()
                if blk == 1:
                    # zero-prime all 4 banks: 0-weight matmuls keep the PE
                    # busy across the block-0 -> block-1 DMA handoff and
                    # leave the banks cleared for accumulation
                    for g in (g0, g1):
                        for m in range(2):
                            nc.tensor.matmul(pss[(g, m)], wsb,
                                             xtiles[1][:, 0],
                                             start=True, stop=False)
                            primed.add((g, m))

                def kloop(g):
                    for m in range(2):
                        for k in range(KM):
                            nc.tensor.matmul(
                                pss[(g, m)], w1w(k, m), xtiles[g][:, k],
                                start=(k == 0 and (g, m) not in primed),
                                stop=False)

                if blk < NBLK - 1:
                    # pack right after the k-loops so the relus start as
                    # early as possible; the deferred layer-2 flushes run
                    # after (their h inputs are then long ready)
                    kloop(g0)
                    kloop(g1)
                    pack_quad(blk, pss)
                    while pend:
                        layer2(*pend.pop(0))
                    pend.append((relu_pair(g0, pss), g0))
                    pend.append((relu_pair(g1, pss), g1))
                else:
                    # last block: per-group tails so the final store chain
                    # is short; g14's relu runs during g15's k-loop
                    kloop(g0)
                    pack_pair(g0, pss)
                    layer2(*pend.pop(0))
                    hs14 = relu_pair(g0, pss)
                    kloop(g1)
                    pack_pair(g1, pss)
                    # final group's relus split across scalar/vector and
                    # emitted BEFORE the layer-2 flushes: they run in
                    # parallel with the other groups' layer-2 matmuls, so
                    # the closing chain is one activation latency
                    h0 = hp.tile([P, NB], bf16, tag="h", name=f"h_{g1}_0")
                    nc.scalar.activation(h0, pss[(g1, 0)], Relu,
                                         bias=b1sb[:, 0:1])
                    h1 = hp.tile([P, NB], bf16, tag="h", name=f"h_{g1}_1")
                    nc.vector.tensor_scalar(h1, pss[(g1, 1)], b1sb[:, 1:2],
                                            0.0, add, amax)
                    layer2(*pend.pop(0))
                    layer2(hs14, g0)
                    layer2([h0, h1], g1)

    nc.compile()
    return nc


def _fold_weights(conv_w, W1):
    """W1eff[784,256] such that x @ W1eff == flatten(conv(x)) @ W1.T."""
    cw = conv_w.astype(np.float64)
    W1r = W1.astype(np.float64).reshape(NF1, 26, 26).transpose(1, 2, 0)
    W1eff = np.zeros((28, 28, NF1), np.float64)
    for dr in range(3):
        for dc in range(3):
            W1eff[dr:dr + 26, dc:dc + 26, :] += cw[dr, dc] * W1r
    return W1eff.reshape(784, NF1)


def _prep_inputs(x, conv_w, W1, b1, W2, b2):
    bf16 = ml_dtypes.bfloat16
    W1eff = _fold_weights(conv_w, W1)
    wpack = np.zeros((P, WCOLS), np.float64)
    wpack[:, :KM * NF1] = W1eff[:768].reshape(KM, P, NF1).transpose(
        1, 0, 2).reshape(P, KM * NF1)
    W1T0 = KM * NF1
    wpack[0:KT, W1T0:W1T0 + P] = W1eff[768:784, 0:128]
    wpack[32:32 + KT, W1T0:W1T0 + P] = W1eff[768:784, 128:256]
    wpack[64:64 + KT, W1T0:W1T0 + P] = W1eff[768:784, 0:128]
    wpack[96:96 + KT, W1T0:W1T0 + P] = W1eff[768:784, 128:256]
    W20 = W1T0 + P
    wpack[:, W20:W20 + 2 * NO] = W2.T.astype(np.float64).reshape(
        2, P, NO).transpose(1, 0, 2).reshape(P, 2 * NO)
    wpack = wpack.astype(bf16)
    b1p = np.ascontiguousarray(b1.astype(np.float32).reshape(2, P).T)
    b2p = b2.astype(np.float32).reshape(NO, 1)

    in_maps = []
    for c in range(NCORES):
        xcT = np.ascontiguousarray(
            x[c * BC:(c + 1) * BC].T).astype(f8)  # [784, BC]
        xmain = np.ascontiguousarray(
            xcT[:768].reshape(KM, P, NGRP, NB).transpose(2, 1, 0, 3)
        ).reshape(NGRP, P, KM * NB)
        xq0p = np.ascontiguousarray(
            xcT[:768, 0:512].reshape(KM, P, 4, 128).transpose(2, 1, 0, 3)
        ).reshape(4, P, KM * 128)
        xtail = np.zeros((NBLK, P, NB), f8)
        tl = xcT[768:784].reshape(KT, NBLK, 2, NB)
        xtail[:, 0:KT] = tl[:, :, 0].transpose(1, 0, 2)
        xtail[:, 32:32 + KT] = tl[:, :, 0].transpose(1, 0, 2)
        xtail[:, 64:64 + KT] = tl[:, :, 1].transpose(1, 0, 2)
        xtail[:, 96:96 + KT] = tl[:, :, 1].transpose(1, 0, 2)
        in_maps.append({
            "xq0": np.ascontiguousarray(xq0p),
            "xb": xmain, "xtl": xtail,
            "wp": wpack, "b1": b1p, "b2": b2p,
        })
    return in_maps


def kernel(x, conv_w, W1, b1, W2, b2, _trace=False, _trace_kwargs=None):
    global _PROG
    from concourse import bass_utils

    x = np.asarray(x, dtype=np.float32)
    conv_w = np.asarray(conv_w, dtype=np.float32)
    W1 = np.asarray(W1, dtype=np.float32)
    b1 = np.asarray(b1, dtype=np.float32)
    W2 = np.asarray(W2, dtype=np.float32)
    b2 = np.asarray(b2, dtype=np.float32)
    assert x.shape == (B, 784), x.shape

    if _PROG is None:
        _PROG = _build_program()

    in_maps = _prep_inputs(x, conv_w, W1, b1, W2, b2)
    kwargs = dict(_trace_kwargs or {})
    res = bass_utils.run_bass_kernel_spmd(
        _PROG, in_maps, core_ids=list(range(NCORES)), trace=_trace, **kwargs)

    out = np.empty((B, NO), np.float32)
    for c in range(NCORES):
        out[c * BC:(c + 1) * BC] = res.results[c]["out"].T
    if _trace:
        return out, res
    return out



# revision 18
# speedup vs baseline: 1.1539x; 1.0421x over previous
"""Trainium2 Bass kernel for nn_DigitConvolutionalModel.

Model: x[B,784] -> conv3x3(valid, 28x28->26x26) -> flatten -> Linear(676,256)
       -> relu -> Linear(256,10).

The conv is linear, so it is folded into the first Linear on the host:
  h_pre = x @ W1eff + b1,  W1eff[784,256] = C @ W1.T  (C = conv as matrix)
leaving a plain 2-layer MLP for the device:
  out = relu(x @ W1eff + b1) @ W2.T + b2

Sharding: pure data parallelism over the batch dim across 8 NeuronCores
(8192 samples/core); weights replicated.

Numerics: x is transported in fp8 E3M4 (4 mantissa bits; the PE streams
fp8 moving data at the same 1 col/cycle as bf16, so this halves HBM
traffic at zero PE cost); weights stay bf16, accumulation fp32 in PSUM.
Measured rel err 0.0132 vs the 0.02 gate.

Schedule: the 784-dim contraction is 6 full 128-row chunks plus a
16-row tail applied via 4 row-tiled matmuls packed into distinct 32-row
PE groups (they execute concurrently, ~4ns apart). Batch groups run in
blocks of 2 (4 layer-1 PSUM banks per block from a 6-bank ps1 pool).
Per block: both k-loops, then the tail pack (so relus start early),
then the previous block's deferred layer-2 flushes (their h inputs are
long ready by then). Engine roles are strict to avoid cross-block FIFO
dependency cycles: scalar = relus only, vector = bias adds only,
sync ring = all x/xt loads (need-order, group 0 in contiguous column
quarters), scalar ring = weight slices in consumption order, gpsimd
SWDGE = interior output stores, final stores on the idle sync ring.
Warmup/filler matmuls keep the PE busy through the startup DMA fill so
the HAM clock gate un-throttles early, and block 1's PSUM banks are
zero-primed to bridge the block-0 -> block-1 handoff.
"""

import sys

if "/opt/trn_rl_repo" not in sys.path:
    sys.path.insert(0, "/opt/trn_rl_repo")

import ml_dtypes
import numpy as np

f8 = ml_dtypes.float8_e3m4

B = 65536
NCORES = 8
BC = B // NCORES  # 8192 samples per core
P = 128
KM = 6            # full 128-row contraction chunks (rows 0..767)
KT = 16           # tail contraction rows (768..783)
NF1 = 256         # layer-1 output features (2 halves of 128)
NO = 10           # logits
NB = 512          # batch columns per matmul group (one PSUM bank, fp32)
NGRP = BC // NB   # 16 groups per core
NBLK = NGRP // 2  # 8 blocks of 2 groups
WCOLS = KM * NF1 + P + 2 * NO  # packed weight columns: w1 | w1tail | w2

_PROG = None


def _build_program():
    import concourse.tile as tile
    from concourse import bacc, mybir

    bf16 = mybir.dt.bfloat16
    f32 = mybir.dt.float32
    Relu = mybir.ActivationFunctionType.Relu
    Identity = mybir.ActivationFunctionType.Identity
    add = mybir.AluOpType.add
    amax = mybir.AluOpType.max

    nc = bacc.Bacc("TRN2", target_bir_lowering=False, debug=False,
                   num_devices=NCORES)
    f8 = mybir.dt.float8e3
    # group-contiguous layouts: each slice below is one fully sequential
    # HBM read (partition-major inside), which keeps the SDMA engines on
    # contiguous streams
    xb = nc.dram_tensor("xb", [NGRP, P, KM * NB], f8,
                        kind="ExternalInput").ap()
    xtl = nc.dram_tensor("xtl", [NBLK, P, NB], f8,
                         kind="ExternalInput").ap()
    wp = nc.dram_tensor("wp", [P, WCOLS], bf16, kind="ExternalInput").ap()
    b1 = nc.dram_tensor("b1", [P, 2], f32, kind="ExternalInput").ap()
    b2 = nc.dram_tensor("b2", [NO, 1], f32, kind="ExternalInput").ap()
    out = nc.dram_tensor("out", [NO, BC], f32, kind="ExternalOutput").ap()

    with tile.TileContext(nc) as tc:
        with (
            tc.tile_pool(name="singles", bufs=1) as singles,
            tc.tile_pool(name="xp", bufs=10) as xp,
            tc.tile_pool(name="xtp", bufs=8) as xtp,
            tc.tile_pool(name="hp", bufs=8) as hp,
            tc.tile_pool(name="op", bufs=5) as op,
            tc.tile_pool(name="ps1", bufs=6, space="PSUM") as ps1p,
            tc.tile_pool(name="ps2", bufs=2, space="PSUM") as ps2p,
        ):
            # warmup tile + matmuls: keep the PE busy from the first cycle
            wsb = singles.tile([P, P], bf16)
            nc.vector.memset(wsb, 0.0)
            wmp = ps1p.tile([32, P], f32, tag="ps1", name="warm")

            def filler(n):
                for i in range(n):
                    nc.tensor.matmul(wmp, wsb[:, :32], wsb,
                                     start=(i == 0), stop=(i == n - 1))

            filler(24)

            # ---- weights on the scalar HWDGE ring, sliced in the exact
            # order the opening matmuls consume them; the second eighth of
            # group 0 also rides this ring so the two rings fill the first
            # PSUM banks in parallel.
            wpsb = singles.tile([P, WCOLS], bf16)
            W1T0 = KM * NF1
            W20 = W1T0 + P

            def w1w(k, m):
                c = k * NF1 + m * P
                return wpsb[:, c:c + P]

            def w2w(m):
                c = W20 + m * NO
                return wpsb[:, c:c + NO]

            nc.scalar.dma_start(out=wpsb[:, 0:512], in_=wp[:, 0:512])
            nc.scalar.dma_start(out=wpsb[:, 512:1024], in_=wp[:, 512:1024])
            nc.scalar.dma_start(out=wpsb[:, 1024:WCOLS],
                                in_=wp[:, 1024:WCOLS])
            b1sb = singles.tile([P, 2], f32)
            nc.scalar.dma_start(out=b1sb, in_=b1)
            b2sb = singles.tile([NO, 1], f32)
            nc.scalar.dma_start(out=b2sb, in_=b2)

            # ---- x split across BOTH hardware DGE rings (sync + scalar):
            # one ring alone saturates at ~95-200 GB/s, below the ~266 GB/s
            # the PE needs at roofline. Group 0 loads k-chunk by k-chunk on
            # sync (65KB each, so its matmuls can start almost immediately),
            # then odd groups ride sync and even groups ride scalar (which
            # first carries the weight prologue).
            xg0sb = singles.tile([P, KM, NB], f8)
            for k in range(KM):
                nc.sync.dma_start(out=xg0sb[:, k],
                                  in_=xb[0][:, k * NB:(k + 1) * NB])
            xtiles = [None] * NGRP
            xttiles = [None] * NBLK

            def load_group(g):
                xg = xp.tile([P, KM, NB], f8, tag="x", name=f"x_{g}")
                ring = nc.sync if g % 2 == 1 else nc.scalar
                ring.dma_start(out=xg, in_=xb[g])
                xtiles[g] = xg

            def load_tail(b):
                # xtp has NBLK bufs (no recycling), so these dma_starts on
                # the scalar queue never block behind a pool wait
                xt = xtp.tile([P, NB], f8, tag="xt", name=f"xt_{b}")
                nc.scalar.dma_start(out=xt, in_=xtl[b])
                xttiles[b] = xt

            load_group(1)
            load_tail(0)
            load_group(2)

            osbs = [None] * NBLK

            def layer2(hs, g):
                blk, j = g // 2, g % 2
                if osbs[blk] is None:
                    osbs[blk] = op.tile([NO, 2, NB], f32, tag="o",
                                        name=f"o_{blk}")
                osb = osbs[blk]
                ps2 = ps2p.tile([NO, NB], f32, tag="ps2", name=f"ps2_{g}")
                for m in range(2):
                    nc.tensor.matmul(ps2, w2w(m), hs[m],
                                     start=(m == 0), stop=(m == 1))
                # bias on vector, relus on scalar: an engine that runs both
                # forms a cross-block dependency cycle (bias waits on L2
                # matmuls which wait on relus queued behind the bias)
                nc.vector.tensor_scalar_add(osb[:, j], ps2, b2sb)
                if blk == NBLK - 1:
                    gs = slice(g * NB, (g + 1) * NB)
                    nc.sync.dma_start(out=out[:, gs], in_=osb[:, j])
                elif j == 1:
                    # interior stores ride the software DGE on the idle
                    # gpsimd queue (one 40KB store per ~5us keeps up)
                    gs = slice(blk * 2 * NB, (blk + 1) * 2 * NB)
                    nc.gpsimd.dma_start(out=out[:, gs], in_=osb)

            def relu_pair(g, pss):
                # both relus on the scalar engine (its only job)
                hs = []
                for m in range(2):
                    h = hp.tile([P, NB], bf16, tag="h", name=f"h_{g}_{m}")
                    nc.scalar.activation(h, pss[(g, m)], Relu,
                                         bias=b1sb[:, m:m + 1])
                    hs.append(h)
                return hs

            def pack_pair(g, pss):
                # 16-row contraction tail for one group: 2 row-tiled
                # matmuls in distinct 32-row PE groups (concurrent)
                xt = xttiles[g // 2]
                base = 0 if g % 2 == 0 else 64
                for m in range(2):
                    rs = slice(base + 32 * m, base + 32 * m + KT)
                    nc.tensor.matmul(pss[(g, m)], wpsb[rs, W1T0:W1T0 + P],
                                     xt[rs], start=False, stop=True,
                                     tile_position=(base + 32 * m, 0))

            def pack_quad(blk, pss):
                g0, g1 = 2 * blk, 2 * blk + 1
                xt = xttiles[blk]
                for r, (g, m) in enumerate(
                        [(g0, 0), (g0, 1), (g1, 0), (g1, 1)]):
                    rs = slice(32 * r, 32 * r + KT)
                    nc.tensor.matmul(pss[(g, m)], wpsb[rs, W1T0:W1T0 + P],
                                     xt[rs], start=False, stop=True,
                                     tile_position=(32 * r, 0))

            pend = []
            # ================= block 0 (groups 0, 1) =================
            pss = {(g, m): ps1p.tile([P, NB], f32, tag="ps1",
                                     name=f"ps1_{g}_{m}")
                   for g in (0, 1) for m in range(2)}

            def mm0k(k):
                for m in range(2):
                    nc.tensor.matmul(pss[(0, m)], w1w(k, m), xg0sb[:, k],
                                     start=(k == 0), stop=False)

            # matmuls ordered by DMA arrival (weights wA=k0k1/wB=k2k3/wC
            # on scalar; x chunks k0..k5 on sync); fillers bridge the known
            # arrival bubbles so the HAM clock gate never sees a PE-idle
            # window
            mm0k(0)
            mm0k(1)                       # wA
            filler(6)
            mm0k(2)
            mm0k(3)                       # wB
            load_group(3)
            load_tail(1)
            filler(6)
            mm0k(4)
            mm0k(5)                       # wC
            filler(4)
            # group 1 whole (N=512)
            for m in range(2):
                for k in range(KM):
                    nc.tensor.matmul(pss[(1, m)], w1w(k, m),
                                     xtiles[1][:, k],
                                     start=(k == 0), stop=False)
            load_group(4)
            load_group(5)
            load_tail(2)
            pack_quad(0, pss)
            pend.append((relu_pair(0, pss), 0))
            pend.append((relu_pair(1, pss), 1))

            # ================= blocks 1..7 =================
            for blk in range(1, NBLK):
                g0, g1 = 2 * blk, 2 * blk + 1
                if 2 * blk + 4 < NGRP:
                    load_group(2 * blk + 4)
                    load_group(2 * blk + 5)
                    if blk + 2 < NBLK:
                        load_tail(blk + 2)

                pss = {(g, m): ps1p.tile([P, NB], f32, tag="ps1",
                                         name=f"ps1_{g}_{m}")
                       for g in (g0, g1) for m in range(2)}
                primed = set()
                if blk == 1:
                    # zero-prime all 4 banks: 0-weight matmuls keep the PE
                    # busy across the block-0 -> block-1 DMA handoff and
                    # leave the banks cleared for accumulation
                    for g in (g0, g1):
                        for m in range(2):
                            nc.tensor.matmul(pss[(g, m)], wsb,
                                             xtiles[1][:, 0],
                                             start=True, stop=False)
                            primed.add((g, m))

                def kloop(g):
                    for m in range(2):
                        for k in range(KM):
                            nc.tensor.matmul(
                                pss[(g, m)], w1w(k, m), xtiles[g][:, k],
                                start=(k == 0 and (g, m) not in primed),
                                stop=False)

                if blk < NBLK - 1:
                    # pack right after the k-loops so the relus start as
                    # early as possible; the deferred layer-2 flushes run
                    # after (their h inputs are then long ready)
                    kloop(g0)
                    kloop(g1)
                    pack_quad(blk, pss)
                    while pend:
                        layer2(*pend.pop(0))
                    pend.append((relu_pair(g0, pss), g0))
                    pend.append((relu_pair(g1, pss), g1))
                else:
                    # last block: per-group tails so the final store chain
                    # is short. Both pending blocks' layer-2s flush BEFORE
                    # g15's k-loop so their vector bias-adds overlap PE
                    # work instead of serializing at the end, and g15's
                    # second relu runs on the (idle) gpsimd engine so the
                    # closing chain is one relu + one L2 + one bias + store.
                    kloop(g0)
                    pack_pair(g0, pss)
                    while pend:
                        layer2(*pend.pop(0))
                    hs14 = relu_pair(g0, pss)
                    kloop(g1)
                    layer2(hs14, g0)
                    pack_pair(g1, pss)
                    h0 = hp.tile([P, NB], bf16, tag="h", name=f"h_{g1}_0")
                    nc.scalar.activation(h0, pss[(g1, 0)], Relu,
                                         bias=b1sb[:, 0:1])
                    h1 = hp.tile([P, NB], bf16, tag="h", name=f"h_{g1}_1")
                    nc.vector.tensor_scalar(h1, pss[(g1, 1)], b1sb[:, 1:2],
                                            0.0, add, amax)
                    # g15's bias-add on the scalar engine (free once h0 is
                    # done) so it doesn't queue behind h1's relu on vector
                    blk15, j15 = g1 // 2, g1 % 2
                    osb15 = osbs[blk15]
                    ps2f = ps2p.tile([NO, NB], f32, tag="ps2",
                                     name=f"ps2_{g1}")
                    for m in range(2):
                        nc.tensor.matmul(ps2f, w2w(m), [h0, h1][m],
                                         start=(m == 0), stop=(m == 1))
                    nc.scalar.activation(osb15[:, j15], ps2f, Identity,
                                         bias=b2sb)
                    gs = slice(g1 * NB, (g1 + 1) * NB)
                    nc.sync.dma_start(out=out[:, gs], in_=osb15[:, j15])

    nc.compile()
    return nc


def _fold_weights(conv_w, W1):
    """W1eff[784,256] such that x @ W1eff == flatten(conv(x)) @ W1.T."""
    cw = conv_w.astype(np.float64)
    W1r = W1.astype(np.float64).reshape(NF1, 26, 26).transpose(1, 2, 0)
    W1eff = np.zeros((28, 28, NF1), np.float64)
    for dr in range(3):
        for dc in range(3):
            W1eff[dr:dr + 26, dc:dc + 26, :] += cw[dr, dc] * W1r
    return W1eff.reshape(784, NF1)


def _prep_inputs(x, conv_w, W1, b1, W2, b2):
    bf16 = ml_dtypes.bfloat16
    W1eff = _fold_weights(conv_w, W1)
    wpack = np.zeros((P, WCOLS), np.float64)
    wpack[:, :KM * NF1] = W1eff[:768].reshape(KM, P, NF1).transpose(
        1, 0, 2).reshape(P, KM * NF1)
    W1T0 = KM * NF1
    wpack[0:KT, W1T0:W1T0 + P] = W1eff[768:784, 0:128]
    wpack[32:32 + KT, W1T0:W1T0 + P] = W1eff[768:784, 128:256]
    wpack[64:64 + KT, W1T0:W1T0 + P] = W1eff[768:784, 0:128]
    wpack[96:96 + KT, W1T0:W1T0 + P] = W1eff[768:784, 128:256]
    W20 = W1T0 + P
    wpack[:, W20:W20 + 2 * NO] = W2.T.astype(np.float64).reshape(
        2, P, NO).transpose(1, 0, 2).reshape(P, 2 * NO)
    wpack = wpack.astype(bf16)
    b1p = np.ascontiguousarray(b1.astype(np.float32).reshape(2, P).T)
    b2p = b2.astype(np.float32).reshape(NO, 1)

    in_maps = []
    for c in range(NCORES):
        xcT = np.ascontiguousarray(
            x[c * BC:(c + 1) * BC].T).astype(f8)  # [784, BC]
        xmain = np.ascontiguousarray(
            xcT[:768].reshape(KM, P, NGRP, NB).transpose(2, 1, 0, 3)
        ).reshape(NGRP, P, KM * NB)
        xtail = np.zeros((NBLK, P, NB), f8)
        tl = xcT[768:784].reshape(KT, NBLK, 2, NB)
        xtail[:, 0:KT] = tl[:, :, 0].transpose(1, 0, 2)
        xtail[:, 32:32 + KT] = tl[:, :, 0].transpose(1, 0, 2)
        xtail[:, 64:64 + KT] = tl[:, :, 1].transpose(1, 0, 2)
        xtail[:, 96:96 + KT] = tl[:, :, 1].transpose(1, 0, 2)
        in_maps.append({
            "xb": xmain, "xtl": xtail,
            "wp": wpack, "b1": b1p, "b2": b2p,
        })
    return in_maps


def kernel(x, conv_w, W1, b1, W2, b2, _trace=False, _trace_kwargs=None):
    global _PROG
    from concourse import bass_utils

    x = np.asarray(x, dtype=np.float32)
    conv_w = np.asarray(conv_w, dtype=np.float32)
    W1 = np.asarray(W1, dtype=np.float32)
    b1 = np.asarray(b1, dtype=np.float32)
    W2 = np.asarray(W2, dtype=np.float32)
    b2 = np.asarray(b2, dtype=np.float32)
    assert x.shape == (B, 784), x.shape

    if _PROG is None:
        _PROG = _build_program()

    in_maps = _prep_inputs(x, conv_w, W1, b1, W2, b2)
    kwargs = dict(_trace_kwargs or {})
    res = bass_utils.run_bass_kernel_spmd(
        _PROG, in_maps, core_ids=list(range(NCORES)), trace=_trace, **kwargs)

    out = np.empty((B, NO), np.float32)
    for c in range(NCORES):
        out[c * BC:(c + 1) * BC] = res.results[c]["out"].T
    if _trace:
        return out, res
    return out

